# revision 14
# baseline (speedup 1.0000x reference)
"""Trainium2 Bass kernel for nn_DiffusionVIProcess (diffusion-VI sampling scan).

Strategy (data-parallel over batch B across 8 NeuronCores, hint-compliant):
  * shard B=2048 into 8 slices of 256; parameters replicated; the scalar
    log_w is an all-reduce (host-side fp64 sum of tiny per-core partials).
  * On-chip layout is transposed + 2-half packed: tiles are [128, 2048] where
    partition p = h*64 + k encodes (row-half h, z-channel k) and the free dim
    is the flattened (S x B_local)/2 row index. The host packs eps / z0 into
    this layout (and unpacks z_samples) during shard/unshard, so every DMA is
    a fully-contiguous 1-2 MiB transfer.
  * Recurrence math is pushed onto the TensorEngine with block-diagonal
    doubled matrices. Two forms:
      form="direct":  psum_z = A2 x z_{t-1} + I2 x CD2 ;
                      z_t = (psum_z + beta_t) + std_t*eps_t     [fused DVE]
      form="incr":    psum_z = G2p x z_{t-1} + I2 x CD2 + I2 x esb_t ;
                      z_t = (psum_z + beta_t) + z_{t-1}          [fused DVE]
    (A = I + dt*W, G2p = dt*W blocks.)  The "incr" form keeps every matmul
    product SMALL relative to the state, so reduced-precision fp32r matmuls
    (1 cycle/row instead of fp32's 4) are numerically safe: the identity
    accumulation z_{t-1} + ... happens exactly, on the VectorEngine.
  * log-weight increment, cancellation-free: with rho = (dt*delta@W - dt*cd)/std,
        bwd_lp - fwd_lp = 0.5*(e^2-d^2), d = rho - e  =>  e^2-d^2 = 2*e*rho - rho^2
        P := std*rho = G2p x z_t + G2n x z_{t-1} + I2 x NCD2    (TensorE)
        sum 2*e*rho : one DVE tensor_tensor_reduce  (accum into SA[:,t])
        sum rho^2   : one ScalarE Square activation (accum into SB[:,t])
    Boundary terms are two more Square-accumulations (z0, zT).
  * Final reduction of the [128,T] partial tiles to the scalar log_w happens
    on the host in fp64 (the "all-reduce sum at the end").
"""

import math

import numpy as np

import concourse.bacc as bacc
import concourse.tile as tile
from concourse import mybir
from concourse.bass_utils import run_bass_kernel_spmd

# Problem constants (hardcoded per contract; see module docstring).
T, S, B, Z, C = 64, 16, 2048, 64, 64
NCORES = 8
BL = B // NCORES          # 256 batch per core
RB = S * BL               # 4096 rows per core
N = RB // 2               # 2048 free elements per half
P = 128                   # partitions
NBANK = N // 512          # matmul free-dim slices (4)
F32 = mybir.dt.float32
F32R = mybir.dt.float32r
ADD = mybir.AluOpType.add
MULT = mybir.AluOpType.mult
SQUARE = mybir.ActivationFunctionType.Square

FORM = "incr"      # "direct" | "incr"
MMDT = "fp32r"     # "fp32" | "fp32r"


def _pack_T2(x_rows_z):
    """[RB, Z] natural -> [128, N] packed-transposed (partition = h*64+k)."""
    return np.ascontiguousarray(
        x_rows_z.reshape(2, N, Z).transpose(0, 2, 1).reshape(P, N)
    )


def _unpack_T2(xT2):
    """[..., 128, N] -> [..., RB, Z] natural."""
    lead = xT2.shape[:-2]
    return (
        xT2.reshape(lead + (2, Z, N))
        .transpose(*range(len(lead)), -3, -1, -2)
        .reshape(lead + (RB, Z))
    )


def _build_program(stds, nsteps, form=FORM, mmdt=MMDT, zs_external=True,
                   repeats=None):
    """Build the per-core Bass program. stds: [nsteps] of sig*sqrt_dt values
    baked in as immediate scale constants. repeats wraps the scan in a
    For_i loop (timing builds).

    With mmdt="fp32r", every matmul operand (weights, eps/cd streams, and the
    running state itself) is declared float32r end-to-end: the host pre-rounds
    the DMA'd values to fp32r (12-bit mantissa) and the state-update DVE op
    writes its output as fp32r, which satisfies the BIR verifier's
    "producer must round" rule with zero extra instructions."""
    nc = bacc.Bacc("TRN2", target_bir_lowering=False, debug=False)

    fp32r = mmdt == "fp32r"
    DS = F32R if fp32r else F32  # dtype of matmul-feeding SBUF tiles

    def fv(x):  # fp32 view of a stream tile (for DVE/ACT consumers)
        return x.bitcast(F32) if fp32r else x

    def dv(x):  # float32r view of an fp32 dram AP (for DMA into DS tiles)
        return x.bitcast(F32R) if fp32r else x

    # DRAM I/O stays float32 at the jax boundary; values are pre-rounded to
    # fp32r representables on the host when mmdt="fp32r".
    eps_d = nc.dram_tensor("eps", [nsteps, P, N], F32, kind="ExternalInput")
    z0_d = nc.dram_tensor("z0t", [P, N], F32, kind="ExternalInput")
    cd2_d = nc.dram_tensor("cd2", [P, N], F32, kind="ExternalInput")
    ncd2_d = nc.dram_tensor("ncd2", [P, N], F32, kind="ExternalInput")
    a2_d = nc.dram_tensor("a2", [P, P], F32, kind="ExternalInput")
    g2p_d = nc.dram_tensor("g2p", [P, P], F32, kind="ExternalInput")
    g2n_d = nc.dram_tensor("g2n", [P, P], F32, kind="ExternalInput")
    i2_d = nc.dram_tensor("i2", [P, P], F32, kind="ExternalInput")
    b2_d = nc.dram_tensor("b2", [P, nsteps], F32, kind="ExternalInput")

    if zs_external:
        zs_d = nc.dram_tensor("zs", [nsteps, P, N], F32, kind="ExternalOutput")
    else:
        zs_d = nc.dram_tensor("zs_scratch", [nsteps, P, N], F32)
    sa_d = nc.dram_tensor("sa", [P, nsteps], F32, kind="ExternalOutput")
    sb_d = nc.dram_tensor("sb", [P, nsteps], F32, kind="ExternalOutput")
    sbnd_d = nc.dram_tensor("sbnd", [P, 2], F32, kind="ExternalOutput")

    with tile.TileContext(nc) as tc:
        with (
            tc.tile_pool(name="consts", bufs=1) as consts,
            tc.tile_pool(name="stats", bufs=1) as stats,
            tc.tile_pool(name="epsp", bufs=3) as epsp,
            tc.tile_pool(name="stagep", bufs=3) as stagep,
            tc.tile_pool(name="zrp", bufs=3) as zrp,
            tc.tile_pool(name="scratch", bufs=1) as scratch,
            tc.tile_pool(name="pzp", bufs=1, space="PSUM") as pzp,
            tc.tile_pool(name="ppp", bufs=1, space="PSUM") as ppp,
        ):
            # ---- constants ----
            a2_t = consts.tile([P, P], DS)
            g2p_t = consts.tile([P, P], DS)
            g2n_t = consts.tile([P, P], DS)
            i2_t = consts.tile([P, P], DS)
            cd2_t = consts.tile([P, N], DS)
            ncd2_t = consts.tile([P, N], DS)
            b2_t = consts.tile([P, nsteps], F32)
            z0_t = consts.tile([P, N], F32)
            nc.sync.dma_start(out=a2_t, in_=dv(a2_d[:]))
            nc.sync.dma_start(out=g2p_t, in_=dv(g2p_d[:]))
            nc.sync.dma_start(out=g2n_t, in_=dv(g2n_d[:]))
            nc.sync.dma_start(out=i2_t, in_=dv(i2_d[:]))
            nc.sync.dma_start(out=cd2_t, in_=dv(cd2_d[:]))
            nc.sync.dma_start(out=ncd2_t, in_=dv(ncd2_d[:]))
            nc.sync.dma_start(out=b2_t, in_=b2_d[:])
            nc.sync.dma_start(out=z0_t, in_=z0_d[:])

            sa_t = stats.tile([P, nsteps], F32)
            sb_t = stats.tile([P, nsteps], F32)
            sbnd_t = stats.tile([P, 2], F32)
            scr_t = scratch.tile([P, N], F32)

            def body(_iv=None):
                # boundary: sum z0^2 per partition
                nc.scalar.activation(
                    out=scr_t, in_=z0_t, func=SQUARE,
                    accum_out=sbnd_t[:, 0:1],
                )
                zprev = z0_t
                if fp32r:
                    # fp32r-rounded view of the state for the TensorEngine
                    zrprev = zrp.tile([P, N], DS, name="zr")
                    nc.scalar.copy(out=zrprev, in_=z0_t)
                else:
                    zrprev = z0_t
                ebig = None
                sbig = None
                for t in range(nsteps):
                    j = t % 2
                    std = float(stds[t])
                    if j == 0:
                        ebig = epsp.tile([P, 2, N], DS, name="ebig")
                        hi = min(t + 2, nsteps)
                        nc.sync.dma_start(
                            out=ebig[:, : hi - t, :],
                            in_=dv(eps_d[t:hi].rearrange("j p n -> p j n")),
                        )
                        sbig = stagep.tile([P, 2, N], F32, name="sbig")
                    esb = ebig[:, j, :]
                    znew = sbig[:, j, :]

                    pz = pzp.tile([P, N], F32, name="pz")
                    pp = ppp.tile([P, N], F32, name="pp")

                    for s in range(NBANK):
                        sl = slice(s * 512, (s + 1) * 512)
                        nc.tensor.matmul(
                            pz[:, sl], i2_t, cd2_t[:, sl],
                            start=True, stop=False,
                        )
                    for s in range(NBANK):
                        sl = slice(s * 512, (s + 1) * 512)
                        nc.tensor.matmul(
                            pp[:, sl], i2_t, ncd2_t[:, sl],
                            start=True, stop=False,
                        )
                    if form == "incr":
                        for s in range(NBANK):
                            sl = slice(s * 512, (s + 1) * 512)
                            nc.tensor.matmul(
                                pz[:, sl], i2_t, esb[:, sl],
                                start=False, stop=False,
                            )
                        for s in range(NBANK):
                            sl = slice(s * 512, (s + 1) * 512)
                            nc.tensor.matmul(
                                pz[:, sl], g2p_t, zrprev[:, sl],
                                start=False, stop=True,
                            )
                        # z_t = (psum_z + dt*b_t) + z_prev   (fused, DVE)
                        nc.vector.scalar_tensor_tensor(
                            out=znew, in0=pz, scalar=b2_t[:, t : t + 1],
                            in1=zprev, op0=ADD, op1=ADD,
                        )
                        if fp32r:
                            zrnew = zrp.tile([P, N], DS, name="zr")
                            nc.scalar.copy(out=zrnew, in_=znew)
                        else:
                            zrnew = znew
                    else:
                        for s in range(NBANK):
                            sl = slice(s * 512, (s + 1) * 512)
                            nc.tensor.matmul(
                                pz[:, sl], a2_t, zrprev[:, sl],
                                start=False, stop=True,
                            )
                        # z_t = (psum_z + dt*b_t) + std*eps   (fused, DVE)
                        nc.vector.scalar_tensor_tensor(
                            out=znew, in0=pz, scalar=b2_t[:, t : t + 1],
                            in1=fv(esb), op0=ADD, op1=ADD,
                        )
                        if fp32r:
                            zrnew = zrp.tile([P, N], DS, name="zr")
                            nc.scalar.copy(out=zrnew, in_=znew)
                        else:
                            zrnew = znew

                    # P = G2p x z_t + G2n x z_prev + I2 x NCD2   (psum_P)
                    for s in range(NBANK):
                        sl = slice(s * 512, (s + 1) * 512)
                        nc.tensor.matmul(
                            pp[:, sl], g2p_t, zrnew[:, sl],
                            start=False, stop=False,
                        )
                    for s in range(NBANK):
                        sl = slice(s * 512, (s + 1) * 512)
                        nc.tensor.matmul(
                            pp[:, sl], g2n_t, zrprev[:, sl],
                            start=False, stop=True,
                        )

                    # SA[:,t] = sum_n (esb * 2/std^2) * P   (= 2*sum e*rho)
                    # (scalar_tensor_tensor with accum_out; tensor_tensor_reduce
                    # fails at runtime on this NRT/axon path)
                    nc.vector.scalar_tensor_tensor(
                        out=esb, in0=fv(esb), scalar=float(2.0 / (std * std)),
                        in1=pp, op0=MULT, op1=MULT,
                        accum_out=sa_t[:, t : t + 1],
                    )
                    # SB[:,t] = sum_n (P/std)^2   (= sum rho^2)
                    nc.scalar.activation(
                        out=scr_t, in_=pp, func=SQUARE, scale=float(1.0 / std),
                        accum_out=sb_t[:, t : t + 1],
                    )

                    if j == 1 or t == nsteps - 1:
                        lo = t - j
                        nc.sync.dma_start(
                            out=zs_d[lo : t + 1].rearrange("j p n -> p j n"),
                            in_=sbig[:, : t + 1 - lo, :],
                        )
                    zprev = znew
                    zrprev = zrnew

                # boundary: sum zT^2 per partition
                nc.scalar.activation(
                    out=scr_t, in_=zprev, func=SQUARE, accum_out=sbnd_t[:, 1:2]
                )

            if repeats is None:
                body()
            else:
                with tc.For_i(0, repeats, 1):
                    body()

            nc.sync.dma_start(out=sa_d[:], in_=sa_t)
            nc.sync.dma_start(out=sb_d[:], in_=sb_t)
            nc.sync.dma_start(out=sbnd_d[:], in_=sbnd_t)

    nc.compile()
    return nc


def _round_fp32r(x):
    """Round fp32 values to fp32r representables (11-bit mantissa, RNE) —
    bit-exact with neuron_dtypes.static_cast_fp32_to_fp32r."""
    u = np.ascontiguousarray(x, np.float32).view(np.uint32)
    u2 = (u + np.uint32(0x7FF) + ((u >> np.uint32(12)) & np.uint32(1))) & np.uint32(
        0xFFFFF000
    )
    return u2.view(np.float32)


def _host_prep(z0, eps, context, W, Wc, b, sigmas, nsteps, mmdt=MMDT):
    """Build per-core in_maps (list of dicts)."""
    dt = np.float32(1.0 / T)
    sqrt_dt = np.float32(math.sqrt(1.0 / T))
    stds = (sigmas.astype(np.float32) * sqrt_dt).astype(np.float32)
    rnd = _round_fp32r if mmdt == "fp32r" else (lambda x: x)

    A = np.eye(Z, dtype=np.float32) + dt * W
    A2 = np.zeros((P, P), np.float32)
    A2[:64, :64] = A
    A2[64:, 64:] = A
    G2p = np.zeros((P, P), np.float32)
    G2p[:64, :64] = dt * W
    G2p[64:, 64:] = dt * W
    G2n = -G2p
    I2 = np.eye(P, dtype=np.float32)
    B2 = np.ascontiguousarray(dt * np.tile(b[:nsteps].T, (2, 1)))  # [128, nsteps]

    in_maps = []
    for core in range(NCORES):
        bsl = slice(core * BL, (core + 1) * BL)
        cd = (context[bsl].astype(np.float32) @ Wc.astype(np.float32)).astype(
            np.float32
        )
        CD2 = np.ascontiguousarray(dt * np.tile(cd.T, (2, N // BL)))  # [128, N]
        z0t = _pack_T2(np.ascontiguousarray(z0[:, bsl, :]).reshape(RB, Z))
        # esb stream: std_t * eps_t, packed  [nsteps, 128, N]
        eps_shard = np.ascontiguousarray(eps[:nsteps, :, bsl, :]).reshape(
            nsteps, RB, Z
        )
        esb = np.empty((nsteps, P, N), np.float32)
        for t in range(nsteps):
            esb[t] = rnd(_pack_T2(eps_shard[t] * stds[t]))
        in_maps.append(
            {
                "eps": esb,
                "z0t": z0t,
                "cd2": rnd(CD2),
                "ncd2": rnd(np.ascontiguousarray(-CD2)),
                "a2": rnd(A2),
                "g2p": rnd(G2p),
                "g2n": rnd(G2n),
                "i2": I2,
                "b2": B2,
            }
        )
    return in_maps, stds


def _finalize(results, z0, nsteps):
    """Gather per-core outputs -> (log_w fp32 scalar, z_samples [T+1,S,B,Z])."""
    zs = np.empty((nsteps + 1, S, B, Z), np.float32)
    zs[0] = z0
    lw = 0.0
    for core, r in enumerate(results):
        bsl = slice(core * BL, (core + 1) * BL)
        zs[1:, :, bsl, :] = _unpack_T2(r["zs"]).reshape(nsteps, S, BL, Z)
        lw += (
            r["sa"].astype(np.float64).sum()
            - r["sb"].astype(np.float64).sum()
            + r["sbnd"][:, 0].astype(np.float64).sum()
            - r["sbnd"][:, 1].astype(np.float64).sum()
        )
    log_w = np.float32(0.5 / S * lw)
    return log_w, zs


_PROGRAM_CACHE = {}


def _get_program(stds, nsteps, form=FORM, mmdt=MMDT):
    key = (nsteps, form, mmdt, tuple(np.asarray(stds, np.float32).tolist()))
    if key not in _PROGRAM_CACHE:
        _PROGRAM_CACHE[key] = _build_program(stds, nsteps, form=form, mmdt=mmdt)
    return _PROGRAM_CACHE[key]


def kernel(z0, eps, context, W, Wc, b, sigmas, _nsteps=T, _form=FORM,
           _mmdt=MMDT):
    z0 = np.asarray(z0, np.float32)
    eps = np.asarray(eps, np.float32)
    in_maps, stds = _host_prep(
        z0, eps, np.asarray(context), np.asarray(W), np.asarray(Wc),
        np.asarray(b), np.asarray(sigmas), _nsteps, mmdt=_mmdt,
    )
    nc = _get_program(stds[:_nsteps], _nsteps, _form, _mmdt)
    out = run_bass_kernel_spmd(nc, in_maps, list(range(NCORES)))
    return _finalize(out.results, z0, _nsteps)


if __name__ == "__main__":
    rng = np.random.default_rng(0)
    inputs = {
        "z0": rng.standard_normal((S, B, Z)).astype(np.float32),
        "eps": rng.standard_normal((T, S, B, Z)).astype(np.float32),
        "context": rng.standard_normal((B, C)).astype(np.float32),
        "W": (rng.standard_normal((Z, Z)) * 0.02).astype(np.float32),
        "Wc": (rng.standard_normal((C, Z)) * 0.02).astype(np.float32),
        "b": (rng.standard_normal((T, Z)) * 0.02).astype(np.float32),
        "sigmas": rng.uniform(0.5, 1.0, (T,)).astype(np.float32),
    }
    log_w, zs = kernel(**inputs)
    print("log_w:", log_w, "zs:", zs.shape, zs.dtype)


# revision 16
# speedup vs baseline: 1.0406x; 1.0406x over previous
"""Trainium2 Bass kernel for nn_DiffusionVIProcess (diffusion-VI sampling scan).

Strategy (data-parallel over batch B across 8 NeuronCores, hint-compliant):
  * shard B=2048 into 8 slices of 256; parameters replicated; the scalar
    log_w is an all-reduce (host-side fp64 sum of tiny per-core partials).
  * On-chip layout is transposed + 2-half packed: tiles are [128, 2048] where
    partition p = h*64 + k encodes (row-half h, z-channel k) and the free dim
    is the flattened (S x B_local)/2 row index. The host packs eps / z0 into
    this layout (and unpacks z_samples) during shard/unshard, so every DMA is
    a fully-contiguous 1-2 MiB transfer.
  * Recurrence math is pushed onto the TensorEngine with block-diagonal
    doubled matrices. Two forms:
      form="direct":  psum_z = A2 x z_{t-1} + I2 x CD2 ;
                      z_t = (psum_z + beta_t) + std_t*eps_t     [fused DVE]
      form="incr":    psum_z = G2p x z_{t-1} + I2 x CD2 + I2 x esb_t ;
                      z_t = (psum_z + beta_t) + z_{t-1}          [fused DVE]
    (A = I + dt*W, G2p = dt*W blocks.)  The "incr" form keeps every matmul
    product SMALL relative to the state, so reduced-precision fp32r matmuls
    (1 cycle/row instead of fp32's 4) are numerically safe: the identity
    accumulation z_{t-1} + ... happens exactly, on the VectorEngine.
  * log-weight increment, cancellation-free: with rho = (dt*delta@W - dt*cd)/std,
        bwd_lp - fwd_lp = 0.5*(e^2-d^2), d = rho - e  =>  e^2-d^2 = 2*e*rho - rho^2
        P := std*rho = G2p x z_t + G2n x z_{t-1} + I2 x NCD2    (TensorE)
        sum 2*e*rho : one DVE tensor_tensor_reduce  (accum into SA[:,t])
        sum rho^2   : one ScalarE Square activation (accum into SB[:,t])
    Boundary terms are two more Square-accumulations (z0, zT).
  * Final reduction of the [128,T] partial tiles to the scalar log_w happens
    on the host in fp64 (the "all-reduce sum at the end").
"""

import math

import numpy as np

import concourse.bacc as bacc
import concourse.tile as tile
from concourse import mybir
from concourse.bass_utils import run_bass_kernel_spmd

# Problem constants (hardcoded per contract; see module docstring).
T, S, B, Z, C = 64, 16, 2048, 64, 64
NCORES = 8
BL = B // NCORES          # 256 batch per core
RB = S * BL               # 4096 rows per core
N = RB // 2               # 2048 free elements per half
P = 128                   # partitions
NBANK = N // 512          # matmul free-dim slices (4)
F32 = mybir.dt.float32
F32R = mybir.dt.float32r
ADD = mybir.AluOpType.add
MULT = mybir.AluOpType.mult
SQUARE = mybir.ActivationFunctionType.Square

FORM = "incr"      # "direct" | "incr"
MMDT = "fp32r"     # "fp32" | "fp32r"
STATE_EXACT = True  # keep fp32 state + separate rounded view for the PE


def _pack_T2(x_rows_z):
    """[RB, Z] natural -> [128, N] packed-transposed (partition = h*64+k)."""
    return np.ascontiguousarray(
        x_rows_z.reshape(2, N, Z).transpose(0, 2, 1).reshape(P, N)
    )


def _unpack_T2(xT2):
    """[..., 128, N] -> [..., RB, Z] natural."""
    lead = xT2.shape[:-2]
    return (
        xT2.reshape(lead + (2, Z, N))
        .transpose(*range(len(lead)), -3, -1, -2)
        .reshape(lead + (RB, Z))
    )


def _build_program(stds, nsteps, form=FORM, mmdt=MMDT, zs_external=True,
                   repeats=None, state_exact=None):
    """Build the per-core Bass program. stds: [nsteps] of sig*sqrt_dt values
    baked in as immediate scale constants. repeats wraps the scan in a
    For_i loop (timing builds).

    With mmdt="fp32r", every matmul operand (weights, eps/cd streams, and the
    running state itself) is declared float32r end-to-end: the host pre-rounds
    the DMA'd values to fp32r (12-bit mantissa) and the state-update DVE op
    writes its output as fp32r, which satisfies the BIR verifier's
    "producer must round" rule with zero extra instructions."""
    nc = bacc.Bacc("TRN2", target_bir_lowering=False, debug=False)

    if state_exact is None:
        state_exact = STATE_EXACT
    fp32r = mmdt == "fp32r"
    round_state = fp32r and state_exact  # extra ACT pass producing the
    # fp32r matmul view; without it the DVE state-update writes fp32r
    # directly (state quantized to 11-bit mantissa each step)
    state_ds = fp32r and not state_exact  # state tiles are fp32r-typed

    def sv(x):  # fp32 view of a STATE tile
        return x.bitcast(F32) if state_ds else x
    DS = F32R if fp32r else F32  # dtype of matmul-feeding SBUF tiles

    def fv(x):  # fp32 view of a stream tile (for DVE/ACT consumers)
        return x.bitcast(F32) if fp32r else x

    def dv(x):  # float32r view of an fp32 dram AP (for DMA into DS tiles)
        return x.bitcast(F32R) if fp32r else x

    # DRAM I/O stays float32 at the jax boundary; values are pre-rounded to
    # fp32r representables on the host when mmdt="fp32r".
    eps_d = nc.dram_tensor("eps", [nsteps, P, N], F32, kind="ExternalInput")
    z0_d = nc.dram_tensor("z0t", [P, N], F32, kind="ExternalInput")
    cd2_d = nc.dram_tensor("cd2", [P, N], F32, kind="ExternalInput")
    ncd2_d = nc.dram_tensor("ncd2", [P, N], F32, kind="ExternalInput")
    a2_d = nc.dram_tensor("a2", [P, P], F32, kind="ExternalInput")
    g2p_d = nc.dram_tensor("g2p", [P, P], F32, kind="ExternalInput")
    g2n_d = nc.dram_tensor("g2n", [P, P], F32, kind="ExternalInput")
    i2_d = nc.dram_tensor("i2", [P, P], F32, kind="ExternalInput")
    b2_d = nc.dram_tensor("b2", [P, nsteps], F32, kind="ExternalInput")

    if zs_external:
        zs_d = nc.dram_tensor("zs", [nsteps, P, N], F32, kind="ExternalOutput")
    else:
        zs_d = nc.dram_tensor("zs_scratch", [nsteps, P, N], F32)
    sa_d = nc.dram_tensor("sa", [P, nsteps], F32, kind="ExternalOutput")
    sb_d = nc.dram_tensor("sb", [P, nsteps], F32, kind="ExternalOutput")
    sbnd_d = nc.dram_tensor("sbnd", [P, 2], F32, kind="ExternalOutput")

    with tile.TileContext(nc) as tc:
        with (
            tc.tile_pool(name="consts", bufs=1) as consts,
            tc.tile_pool(name="stats", bufs=1) as stats,
            tc.tile_pool(name="epsp", bufs=3) as epsp,
            tc.tile_pool(name="stagep", bufs=3) as stagep,
            tc.tile_pool(name="zrp", bufs=3) as zrp,
            tc.tile_pool(name="scratch", bufs=1) as scratch,
            tc.tile_pool(name="pzp", bufs=1, space="PSUM") as pzp,
            tc.tile_pool(name="ppp", bufs=1, space="PSUM") as ppp,
        ):
            # ---- constants ----
            a2_t = consts.tile([P, P], DS)
            g2p_t = consts.tile([P, P], DS)
            g2n_t = consts.tile([P, P], DS)
            i2_t = consts.tile([P, P], DS)
            cd2_t = consts.tile([P, N], DS)
            ncd2_t = consts.tile([P, N], DS)
            b2_t = consts.tile([P, nsteps], F32)
            z0_t = consts.tile([P, N], F32)
            nc.sync.dma_start(out=a2_t, in_=dv(a2_d[:]))
            nc.sync.dma_start(out=g2p_t, in_=dv(g2p_d[:]))
            nc.sync.dma_start(out=g2n_t, in_=dv(g2n_d[:]))
            nc.sync.dma_start(out=i2_t, in_=dv(i2_d[:]))
            nc.sync.dma_start(out=cd2_t, in_=dv(cd2_d[:]))
            nc.sync.dma_start(out=ncd2_t, in_=dv(ncd2_d[:]))
            nc.sync.dma_start(out=b2_t, in_=b2_d[:])
            nc.sync.dma_start(out=z0_t, in_=z0_d[:])

            sa_t = stats.tile([P, nsteps], F32)
            sb_t = stats.tile([P, nsteps], F32)
            sbnd_t = stats.tile([P, 2], F32)
            scr_t = scratch.tile([P, N], F32)

            def body(_iv=None):
                # boundary: sum z0^2 per partition
                nc.scalar.activation(
                    out=scr_t, in_=z0_t, func=SQUARE,
                    accum_out=sbnd_t[:, 0:1],
                )
                zprev = z0_t
                if round_state:
                    # fp32r-rounded view of the state for the TensorEngine
                    zrprev = zrp.tile([P, N], DS, name="zr")
                    nc.scalar.copy(out=zrprev, in_=z0_t)
                elif fp32r:
                    zrprev = zrp.tile([P, N], DS, name="zr")
                    nc.scalar.copy(out=zrprev, in_=z0_t)
                else:
                    zrprev = z0_t
                ebig = None
                sbig = None
                for t in range(nsteps):
                    j = t % 2
                    std = float(stds[t])
                    if j == 0:
                        ebig = epsp.tile([P, 2, N], DS, name="ebig")
                        hi = min(t + 2, nsteps)
                        nc.sync.dma_start(
                            out=ebig[:, : hi - t, :],
                            in_=dv(eps_d[t:hi].rearrange("j p n -> p j n")),
                        )
                        sbig = stagep.tile([P, 2, N], F32 if state_exact else DS, name="sbig")
                    esb = ebig[:, j, :]
                    znew = sbig[:, j, :]

                    pz = pzp.tile([P, N], F32, name="pz")
                    pp = ppp.tile([P, N], F32, name="pp")

                    for s in range(NBANK):
                        sl = slice(s * 512, (s + 1) * 512)
                        nc.tensor.matmul(
                            pz[:, sl], i2_t, cd2_t[:, sl],
                            start=True, stop=False,
                        )
                    for s in range(NBANK):
                        sl = slice(s * 512, (s + 1) * 512)
                        nc.tensor.matmul(
                            pp[:, sl], i2_t, ncd2_t[:, sl],
                            start=True, stop=False,
                        )
                    if form == "incr":
                        for s in range(NBANK):
                            sl = slice(s * 512, (s + 1) * 512)
                            nc.tensor.matmul(
                                pz[:, sl], i2_t, esb[:, sl],
                                start=False, stop=False,
                            )
                        for s in range(NBANK):
                            sl = slice(s * 512, (s + 1) * 512)
                            nc.tensor.matmul(
                                pz[:, sl], g2p_t, zrprev[:, sl],
                                start=False, stop=True,
                            )
                        # z_t = (psum_z + dt*b_t) + z_prev   (fused, DVE)
                        nc.vector.scalar_tensor_tensor(
                            out=znew, in0=pz, scalar=b2_t[:, t : t + 1],
                            in1=sv(zprev), op0=ADD, op1=ADD,
                        )
                        if round_state:
                            zrnew = zrp.tile([P, N], DS, name="zr")
                            nc.scalar.copy(out=zrnew, in_=znew)
                        else:
                            zrnew = znew
                    else:
                        for s in range(NBANK):
                            sl = slice(s * 512, (s + 1) * 512)
                            nc.tensor.matmul(
                                pz[:, sl], a2_t, zrprev[:, sl],
                                start=False, stop=True,
                            )
                        # z_t = (psum_z + dt*b_t) + std*eps   (fused, DVE)
                        nc.vector.scalar_tensor_tensor(
                            out=znew, in0=pz, scalar=b2_t[:, t : t + 1],
                            in1=fv(esb), op0=ADD, op1=ADD,
                        )
                        if round_state:
                            zrnew = zrp.tile([P, N], DS, name="zr")
                            nc.scalar.copy(out=zrnew, in_=znew)
                        else:
                            zrnew = znew

                    # P = G2p x z_t + G2n x z_prev + I2 x NCD2   (psum_P)
                    for s in range(NBANK):
                        sl = slice(s * 512, (s + 1) * 512)
                        nc.tensor.matmul(
                            pp[:, sl], g2p_t, zrnew[:, sl],
                            start=False, stop=False,
                        )
                    for s in range(NBANK):
                        sl = slice(s * 512, (s + 1) * 512)
                        nc.tensor.matmul(
                            pp[:, sl], g2n_t, zrprev[:, sl],
                            start=False, stop=True,
                        )

                    # SA[:,t] = sum_n (esb * 2/std^2) * P   (= 2*sum e*rho)
                    # (scalar_tensor_tensor with accum_out; tensor_tensor_reduce
                    # fails at runtime on this NRT/axon path)
                    nc.vector.scalar_tensor_tensor(
                        out=esb, in0=fv(esb), scalar=float(2.0 / (std * std)),
                        in1=pp, op0=MULT, op1=MULT,
                        accum_out=sa_t[:, t : t + 1],
                    )
                    # SB[:,t] = sum_n (P/std)^2   (= sum rho^2)
                    nc.scalar.activation(
                        out=scr_t, in_=pp, func=SQUARE, scale=float(1.0 / std),
                        accum_out=sb_t[:, t : t + 1],
                    )

                    if j == 1 or t == nsteps - 1:
                        lo = t - j
                        nc.sync.dma_start(
                            out=zs_d[lo : t + 1].rearrange("j p n -> p j n"),
                            in_=sv(sbig[:, : t + 1 - lo, :]),
                        )
                    zprev = znew
                    zrprev = zrnew

                # boundary: sum zT^2 per partition
                nc.scalar.activation(
                    out=scr_t, in_=sv(zprev), func=SQUARE,
                    accum_out=sbnd_t[:, 1:2],
                )

            if repeats is None:
                body()
            else:
                with tc.For_i(0, repeats, 1):
                    body()

            nc.sync.dma_start(out=sa_d[:], in_=sa_t)
            nc.sync.dma_start(out=sb_d[:], in_=sb_t)
            nc.sync.dma_start(out=sbnd_d[:], in_=sbnd_t)

    nc.compile()
    return nc


def _round_fp32r(x):
    """Round fp32 values to fp32r representables (11-bit mantissa, RNE) —
    bit-exact with neuron_dtypes.static_cast_fp32_to_fp32r."""
    u = np.ascontiguousarray(x, np.float32).view(np.uint32)
    u2 = (u + np.uint32(0x7FF) + ((u >> np.uint32(12)) & np.uint32(1))) & np.uint32(
        0xFFFFF000
    )
    return u2.view(np.float32)


def _host_prep(z0, eps, context, W, Wc, b, sigmas, nsteps, mmdt=MMDT):
    """Build per-core in_maps (list of dicts)."""
    dt = np.float32(1.0 / T)
    sqrt_dt = np.float32(math.sqrt(1.0 / T))
    stds = (sigmas.astype(np.float32) * sqrt_dt).astype(np.float32)
    rnd = _round_fp32r if mmdt == "fp32r" else (lambda x: x)

    A = np.eye(Z, dtype=np.float32) + dt * W
    A2 = np.zeros((P, P), np.float32)
    A2[:64, :64] = A
    A2[64:, 64:] = A
    G2p = np.zeros((P, P), np.float32)
    G2p[:64, :64] = dt * W
    G2p[64:, 64:] = dt * W
    G2n = -G2p
    I2 = np.eye(P, dtype=np.float32)
    B2 = np.ascontiguousarray(dt * np.tile(b[:nsteps].T, (2, 1)))  # [128, nsteps]

    in_maps = []
    for core in range(NCORES):
        bsl = slice(core * BL, (core + 1) * BL)
        cd = (context[bsl].astype(np.float32) @ Wc.astype(np.float32)).astype(
            np.float32
        )
        CD2 = np.ascontiguousarray(dt * np.tile(cd.T, (2, N // BL)))  # [128, N]
        z0t = _pack_T2(np.ascontiguousarray(z0[:, bsl, :]).reshape(RB, Z))
        # esb stream: std_t * eps_t, packed  [nsteps, 128, N]
        eps_shard = np.ascontiguousarray(eps[:nsteps, :, bsl, :]).reshape(
            nsteps, RB, Z
        )
        esb = np.empty((nsteps, P, N), np.float32)
        for t in range(nsteps):
            esb[t] = rnd(_pack_T2(eps_shard[t] * stds[t]))
        in_maps.append(
            {
                "eps": esb,
                "z0t": z0t,
                "cd2": rnd(CD2),
                "ncd2": rnd(np.ascontiguousarray(-CD2)),
                "a2": rnd(A2),
                "g2p": rnd(G2p),
                "g2n": rnd(G2n),
                "i2": I2,
                "b2": B2,
            }
        )
    return in_maps, stds


def _finalize(results, z0, nsteps):
    """Gather per-core outputs -> (log_w fp32 scalar, z_samples [T+1,S,B,Z])."""
    zs = np.empty((nsteps + 1, S, B, Z), np.float32)
    zs[0] = z0
    lw = 0.0
    for core, r in enumerate(results):
        bsl = slice(core * BL, (core + 1) * BL)
        zs[1:, :, bsl, :] = _unpack_T2(r["zs"]).reshape(nsteps, S, BL, Z)
        lw += (
            r["sa"].astype(np.float64).sum()
            - r["sb"].astype(np.float64).sum()
            + r["sbnd"][:, 0].astype(np.float64).sum()
            - r["sbnd"][:, 1].astype(np.float64).sum()
        )
    log_w = np.float32(0.5 / S * lw)
    return log_w, zs


_PROGRAM_CACHE = {}


def _get_program(stds, nsteps, form=FORM, mmdt=MMDT):
    key = (nsteps, form, mmdt, tuple(np.asarray(stds, np.float32).tolist()))
    if key not in _PROGRAM_CACHE:
        _PROGRAM_CACHE[key] = _build_program(stds, nsteps, form=form, mmdt=mmdt)
    return _PROGRAM_CACHE[key]


def kernel(z0, eps, context, W, Wc, b, sigmas, _nsteps=T, _form=FORM,
           _mmdt=MMDT):
    z0 = np.asarray(z0, np.float32)
    eps = np.asarray(eps, np.float32)
    in_maps, stds = _host_prep(
        z0, eps, np.asarray(context), np.asarray(W), np.asarray(Wc),
        np.asarray(b), np.asarray(sigmas), _nsteps, mmdt=_mmdt,
    )
    nc = _get_program(stds[:_nsteps], _nsteps, _form, _mmdt)
    out = run_bass_kernel_spmd(nc, in_maps, list(range(NCORES)))
    return _finalize(out.results, z0, _nsteps)


if __name__ == "__main__":
    rng = np.random.default_rng(0)
    inputs = {
        "z0": rng.standard_normal((S, B, Z)).astype(np.float32),
        "eps": rng.standard_normal((T, S, B, Z)).astype(np.float32),
        "context": rng.standard_normal((B, C)).astype(np.float32),
        "W": (rng.standard_normal((Z, Z)) * 0.02).astype(np.float32),
        "Wc": (rng.standard_normal((C, Z)) * 0.02).astype(np.float32),
        "b": (rng.standard_normal((T, Z)) * 0.02).astype(np.float32),
        "sigmas": rng.uniform(0.5, 1.0, (T,)).astype(np.float32),
    }
    log_w, zs = kernel(**inputs)
    print("log_w:", log_w, "zs:", zs.shape, zs.dtype)
